# revision 2
# baseline (speedup 1.0000x reference)
"""Trainium2 Bass kernel for nn_DigitConvolutionalModel — v6.

Model: x[B,784] -> conv3x3(valid) -> flatten -> Linear(676,256) -> relu
       -> Linear(256,10); conv folded into the first Linear on the host:
  out = relu(x @ W1eff + b1) @ W2.T + b2.

Sharding: pure data parallelism over batch across 8 cores (8192/core),
weights replicated; bf16 compute, fp32 PSUM accumulation.

v3 vs the original:
- no PE warmup: the first layer-1 matmul issues as soon as w1's first
  k-chunk (64 KB) and x group 0's first column-half land; the HAM ramp
  runs over real work instead of dummy matmuls
- setup DMA order puts the whole w1 bulk ahead of the tiny bias/w2
  transfers (each dma_start costs ~0.7 us of queue time); b1/b2 ride in
  one packed tensor
- layer-2's two M=10 matmuls are column-tiled to array col-groups 0/1
  (tile_position) so they run concurrently; the two PSUM slices are
  merged during the bias add (scalar_tensor_tensor)
- outputs accumulate in one [10, 8192] SBUF buffer, stored in 4 chunks
  (16 -> 4 store DMAs; every DMA adds ~115 ns to the end-of-kernel
  semaphore drain, which counts in exec time)
"""

import sys

if "/opt/trn_rl_repo" not in sys.path:
    sys.path.insert(0, "/opt/trn_rl_repo")

import ml_dtypes
import numpy as np

B = 65536
NCORES = 8
BC = B // NCORES  # 8192 samples per core
P = 128
KC = 7            # contraction chunks of 128 (784 zero-padded to 896)
NF1 = 256         # layer-1 output features (2 halves of 128)
NO = 10           # logits
NB = 512          # batch columns per matmul group (one PSUM bank, fp32)
NGRP = BC // NB   # 16 groups per core

_PROG = None


def _build_program():
    import concourse.tile as tile
    from concourse import bacc, mybir

    bf16 = mybir.dt.bfloat16
    f32 = mybir.dt.float32

    nc = bacc.Bacc("TRN2", target_bir_lowering=False, debug=False,
                   num_devices=NCORES)
    # groups 0/1 as four contiguous column-halves (wire-speed DMA for the
    # startup-critical loads); groups 2..15 in the group-major layout
    xh = nc.dram_tensor("xh", [P, 4, KC, NB // 2], bf16,
                        kind="ExternalInput").ap()
    xt = nc.dram_tensor("xt", [P, NGRP - 2, KC, NB], bf16,
                        kind="ExternalInput").ap()
    w1 = nc.dram_tensor("w1", [P, KC, NF1], bf16, kind="ExternalInput").ap()
    w2 = nc.dram_tensor("w2", [P, 2, NO], bf16, kind="ExternalInput").ap()
    # bias pack: [:, 0:2] = b1 (2 halves), [0:10, 2] = b2
    bia = nc.dram_tensor("bia", [P, 3], f32, kind="ExternalInput").ap()
    out = nc.dram_tensor("out", [NO, BC], f32, kind="ExternalOutput").ap()

    with tile.TileContext(nc) as tc:
        with (
            tc.tile_pool(name="singles", bufs=1) as singles,
            tc.tile_pool(name="xp", bufs=6) as xp,
            tc.tile_pool(name="xhp", bufs=4) as xhp,
            tc.tile_pool(name="hp", bufs=8) as hp,
            tc.tile_pool(name="op", bufs=3) as op,
            tc.tile_pool(name="ps1", bufs=5, space="PSUM") as ps1p,
            tc.tile_pool(name="ps2", bufs=3, space="PSUM") as ps2p,
        ):
            # w1 as ONE contiguous transfer on the sync ring (the scalar
            # ring and strided slices both run ~3-4x slower)
            w1sb = singles.tile([P, KC, NF1], bf16)
            nc.sync.dma_start(out=w1sb, in_=w1)
            biasb = singles.tile([P, 3], f32)
            nc.scalar.dma_start(out=biasb, in_=bia)
            w2sb = singles.tile([P, 2, NO], bf16)
            nc.scalar.dma_start(out=w2sb, in_=w2)
            b2sb = biasb[0:NO, 2:3]

            osb = singles.tile([NO, BC], f32)
            OSTORE = BC // 4
            state = {"stored": 0}

            # bridge the HAM activity window while the first transfers land:
            # ~24 cold N=128 matmuls ~= 2.6 us of PE busy from ~6.9 us
            wsb = singles.tile([P, P], bf16)
            nc.vector.memset(wsb, 0.0)
            wps = ps2p.tile([42, P], f32, tag="ps2", name="warm")
            NWARM = 24
            for i in range(NWARM):
                nc.tensor.matmul(wps[0:32], wsb[:, :32], wsb,
                                 start=(i == 0), stop=(i == NWARM - 1))

            def layer2(hs, g):
                gs = slice(g * NB, (g + 1) * NB)
                # two col-tiled matmuls on array col-groups 0/1 run
                # concurrently; merge the PSUM slices in the bias add
                ps2 = ps2p.tile([42, NB], f32, tag="ps2", name=f"ps2_{g}")
                nc.tensor.matmul(ps2[0:NO], w2sb[:, 0, :], hs[0],
                                 start=True, stop=True,
                                 tile_position=(0, 0))
                nc.tensor.matmul(ps2[32:32 + NO], w2sb[:, 1, :], hs[1],
                                 start=True, stop=True,
                                 tile_position=(0, 32))
                # one-PSUM-input rule: move the col-group-1 slice (+b2) to
                # SBUF on the scalar engine, then merge on the vector engine
                o2 = op.tile([NO, NB], f32, tag="o2", name=f"o2_{g}")
                nc.scalar.activation(o2, ps2[32:32 + NO],
                                     mybir.ActivationFunctionType.Copy)
                nc.vector.scalar_tensor_tensor(
                    osb[:, gs], ps2[0:NO], b2sb, o2,
                    mybir.AluOpType.add, mybir.AluOpType.add)
                done = (g + 1) * NB
                tgt = state["stored"] + OSTORE
                if done >= tgt and state["stored"] < tgt <= BC - OSTORE:
                    s = slice(state["stored"], tgt)
                    nc.scalar.dma_start(out=out[:, s], in_=osb[:, s])
                    state["stored"] = tgt

            pend = []
            # head halves issued up front on the sync ring, right after w1
            xhs = []
            for i in range(4):
                xa = xhp.tile([P, KC, NB // 2], bf16, tag="xh",
                              name=f"xh_{i}")
                nc.sync.dma_start(out=xa, in_=xh[:, i])
                xhs.append(xa)
            for g in range(NGRP - 1):
                if g <= 1:
                    pss = [ps1p.tile([P, NB], f32, tag="ps1",
                                     name=f"ps1_{g}_{m}") for m in range(2)]
                    for h2 in range(2):
                        hsl = slice(h2 * NB // 2, (h2 + 1) * NB // 2)
                        xa = xhs[2 * g + h2]
                        for k in range(KC):
                            for m in range(2):
                                nc.tensor.matmul(
                                    pss[m][:, hsl],
                                    w1sb[:, k, m * P:(m + 1) * P],
                                    xa[:, k],
                                    start=(k == 0),
                                    stop=(k == KC - 1),
                                )
                    hs = []
                    for m in range(2):
                        h = hp.tile([P, NB], bf16, tag="h", name=f"h_{g}_{m}")
                        nc.scalar.activation(
                            h, pss[m], mybir.ActivationFunctionType.Relu,
                            bias=biasb[:, m:m + 1],
                        )
                        hs.append(h)
                    if g == 0:
                        layer2(hs, g)
                    else:
                        pend.append((hs, g))
                    continue
                xg = xp.tile([P, KC, NB], bf16, tag="x", name=f"x_{g}")
                nc.sync.dma_start(out=xg, in_=xt[:, g - 2])

                pss = [ps1p.tile([P, NB], f32, tag="ps1", name=f"ps1_{g}_{m}")
                       for m in range(2)]
                for k in range(KC):
                    for m in range(2):
                        nc.tensor.matmul(
                            pss[m],
                            w1sb[:, k, m * P:(m + 1) * P],
                            xg[:, k, :],
                            start=(k == 0),
                            stop=(k == KC - 1),
                        )

                if len(pend) == 2:
                    layer2(*pend.pop(0))

                hs = []
                for m in range(2):
                    h = hp.tile([P, NB], bf16, tag="h", name=f"h_{g}_{m}")
                    nc.scalar.activation(
                        h, pss[m], mybir.ActivationFunctionType.Relu,
                        bias=biasb[:, m:m + 1],
                    )
                    hs.append(h)
                pend.append((hs, g))

            # last group: two 256-column halves; relu split across the
            # scalar and vector engines to shorten the exposed tail chain
            gl = NGRP - 1
            NH = NB // 2
            xg = xp.tile([P, KC, NB], bf16, tag="x", name=f"x_{gl}")
            nc.sync.dma_start(out=xg[:, :, :NH], in_=xt[:, gl - 2, :, :NH])
            nc.sync.dma_start(out=xg[:, :, NH:], in_=xt[:, gl - 2, :, NH:])
            for sub in range(2):
                cs = slice(sub * NH, (sub + 1) * NH)
                pss = [ps1p.tile([P, NH], f32, tag="ps1",
                                 name=f"ps1_{gl}_{sub}_{m}") for m in range(2)]
                for k in range(KC):
                    for m in range(2):
                        nc.tensor.matmul(
                            pss[m],
                            w1sb[:, k, m * P:(m + 1) * P],
                            xg[:, k, cs],
                            start=(k == 0),
                            stop=(k == KC - 1),
                        )
                if pend:
                    layer2(*pend.pop(0))
                h0 = hp.tile([P, NH], bf16, tag="h", name=f"h_{gl}_{sub}_0")
                nc.scalar.activation(h0, pss[0],
                                     mybir.ActivationFunctionType.Relu,
                                     bias=biasb[:, 0:1])
                h1 = hp.tile([P, NH], bf16, tag="h", name=f"h_{gl}_{sub}_1")
                nc.vector.tensor_scalar(h1, pss[1], biasb[:, 1:2], 0.0,
                                        mybir.AluOpType.add,
                                        mybir.AluOpType.max)
                ps2 = ps2p.tile([42, NH], f32, tag="ps2",
                                name=f"ps2_{gl}_{sub}")
                nc.tensor.matmul(ps2[0:NO], w2sb[:, 0, :], h0,
                                 start=True, stop=True,
                                 tile_position=(0, 0))
                nc.tensor.matmul(ps2[32:32 + NO], w2sb[:, 1, :], h1,
                                 start=True, stop=True,
                                 tile_position=(0, 32))
                o2 = op.tile([NO, NH], f32, tag="o2", name=f"o2_{gl}_{sub}")
                nc.scalar.activation(o2, ps2[32:32 + NO],
                                     mybir.ActivationFunctionType.Copy)
                nc.vector.scalar_tensor_tensor(
                    osb[:, gl * NB + sub * NH:gl * NB + (sub + 1) * NH],
                    ps2[0:NO], b2sb, o2,
                    mybir.AluOpType.add, mybir.AluOpType.add)
                if sub == 0:
                    # cols 6144:7168 (groups 12-13) are complete here
                    nc.sync.dma_start(
                        out=out[:, BC - OSTORE:BC - 2 * NB],
                        in_=osb[:, BC - OSTORE:BC - 2 * NB])
            nc.sync.dma_start(out=out[:, BC - 2 * NB:],
                              in_=osb[:, BC - 2 * NB:])

    nc.compile()
    return nc


def _fold_weights(conv_w, W1):
    """W1eff[784,256] such that x @ W1eff == flatten(conv(x)) @ W1.T."""
    cw = conv_w.astype(np.float64)
    W1r = W1.astype(np.float64).reshape(NF1, 26, 26).transpose(1, 2, 0)
    W1eff = np.zeros((28, 28, NF1), np.float64)
    for dr in range(3):
        for dc in range(3):
            W1eff[dr:dr + 26, dc:dc + 26, :] += cw[dr, dc] * W1r
    return W1eff.reshape(784, NF1)


def _prep_inputs(x, conv_w, W1, b1, W2, b2):
    bf16 = ml_dtypes.bfloat16
    W1eff = _fold_weights(conv_w, W1)
    w1p = np.zeros((KC * P, NF1), np.float64)
    w1p[:784] = W1eff
    w1p = np.ascontiguousarray(
        w1p.reshape(KC, P, NF1).transpose(1, 0, 2)).astype(bf16)
    w2p = np.ascontiguousarray(
        W2.T.astype(np.float32).reshape(2, P, NO).transpose(1, 0, 2)).astype(bf16)
    biap = np.zeros((P, 3), np.float32)
    biap[:, :2] = b1.astype(np.float32).reshape(2, P).T
    biap[:NO, 2] = b2.astype(np.float32)

    in_maps = []
    for c in range(NCORES):
        xc = np.zeros((KC * P, BC), bf16)
        xcT = np.ascontiguousarray(x[c * BC:(c + 1) * BC].T)
        xc[:784] = xcT.astype(bf16)
        xdev = xc.reshape(KC, P, NGRP, NB).transpose(1, 2, 0, 3)
        # head: groups 0/1 as 4 contiguous column-halves [P, 4, KC, 256]
        xhd = np.ascontiguousarray(
            xdev[:, :2].reshape(P, 2, KC, 2, NB // 2)
            .transpose(0, 1, 3, 2, 4).reshape(P, 4, KC, NB // 2))
        in_maps.append({
            "xh": xhd,
            "xt": np.ascontiguousarray(xdev[:, 2:]),
            "w1": w1p, "w2": w2p, "bia": biap,
        })
    return in_maps


def kernel(x, conv_w, W1, b1, W2, b2, _trace=False, _trace_kwargs=None):
    global _PROG
    from concourse import bass_utils

    x = np.asarray(x, dtype=np.float32)
    conv_w = np.asarray(conv_w, dtype=np.float32)
    W1 = np.asarray(W1, dtype=np.float32)
    b1 = np.asarray(b1, dtype=np.float32)
    W2 = np.asarray(W2, dtype=np.float32)
    b2 = np.asarray(b2, dtype=np.float32)
    assert x.shape == (B, 784), x.shape

    if _PROG is None:
        _PROG = _build_program()

    in_maps = _prep_inputs(x, conv_w, W1, b1, W2, b2)
    kwargs = dict(_trace_kwargs or {})
    res = bass_utils.run_bass_kernel_spmd(
        _PROG, in_maps, core_ids=list(range(NCORES)), trace=_trace, **kwargs)

    out = np.empty((B, NO), np.float32)
    for c in range(NCORES):
        out[c * BC:(c + 1) * BC] = res.results[c]["out"].T
    if _trace:
        return out, res
    return out
